# revision 10
# baseline (speedup 1.0000x reference)
"""CenterLoss kernel for Trainium2 (8 NeuronCores, data-parallel over batch).

reference:  mean(clip(rowsum((x - labels @ centers)^2), 1e-12, 1e12))
labels are exact one-hot rows, so labels @ centers is an embedding gather:
    idx[b]  = max_index(labels[b, :])           (DVE max_index, query = 1.0)
    c[b]    = centers[idx[b], :]                (indirect DMA row gather)
    ps[b]   = rowsum((x[b] - c[b])^2)           (DVE sub -> bf16, ACT square+accum)
Per-core output is a [128, 9] tile of per-sample sums (tile 7 split in two
halves for a shorter tail); the host merges the halves, applies the clip
(never binding for this data, but exact) and takes the mean.

Phase schedule (SWDGE + HWDGE queues running concurrently drop aggregate
DMA from ~430 to ~320 GB/s, so the phases are serialized):
  1. labels on the sync HWDGE ring (3MB, lands ~14.5us)
  2. index chain (DVE FIND_INDEX8) + row gathers on the SWDGE queue
     (8MB, solo, lands ~33us)
  3. x on the sync ring (8MB, solo), gated behind the last gather by a
     WAW write into xbig; 1MB chunks so sub/ACT pipeline per tile
  4. sub -> square+accum trail the x chunks; out store on the scalar
     ring right after the last accumulator read (same-engine ordering)
"""

import numpy as np

import concourse.bacc as bacc
import concourse.bass as bass
import concourse.mybir as mybir
from concourse.tile import TileContext
from concourse.bass_utils import run_bass_kernel_spmd

F32 = mybir.dt.float32
BF16 = mybir.dt.bfloat16
U32 = mybir.dt.uint32

NCORES = 8
B = 8192          # full batch
C = 751           # num classes
D = 2048          # feature dim
BS = B // NCORES  # batch per core = 1024
P = 128           # partitions
NT = BS // P      # batch tiles per core = 8
NACC = NT + 1     # tile 7 contributes two half-sums

CLIP_LO, CLIP_HI = 1e-12, 1e12


def build_nc():
    nc = bacc.Bacc(
        "TRN2",
        target_bir_lowering=False,
        debug=False,
        num_devices=NCORES,
    )
    x = nc.dram_tensor("x", [BS, D], F32, kind="ExternalInput")
    labels = nc.dram_tensor("labels", [BS, C], F32, kind="ExternalInput")
    centers = nc.dram_tensor("centers", [C, D], F32, kind="ExternalInput")
    out = nc.dram_tensor("out", [P, NACC], F32, kind="ExternalOutput")

    with TileContext(nc) as tc:
        with (
            tc.tile_pool(name="big", bufs=1) as bigpool,
            tc.tile_pool(name="small", bufs=1) as spool,
        ):
            ones = spool.tile([P, 8], F32)
            idxs = spool.tile([P, NT, 8], U32)
            acc = spool.tile([P, NACC], F32)
            dif_a = spool.tile([P, D], BF16)
            dif_b = spool.tile([P, D], BF16)
            dsq = spool.tile([P, D], BF16)
            lbig = bigpool.tile([P, NT, C], F32)
            xbig = bigpool.tile([P, NT, D], F32)
            ctile = bigpool.tile([P, NT, D], F32)

            nc.vector.memset(ones[:], 1.0)

            labels_r = labels.rearrange("(n p) c -> p n c", p=P)
            x_r = x.rearrange("(n p) d -> p n d", p=P)

            # labels on the sync HWDGE ring: solo while the SWDGE queue
            # warms up, and HWDGE completion latency is ~3us shorter, so
            # the index chain starts early
            for g in range(2):
                nc.sync.dma_start(
                    out=lbig[:, 4 * g:4 * g + 4, :],
                    in_=labels_r[:, 4 * g:4 * g + 4, :],
                )

            # one-hot -> index (DVE)
            for n in range(NT):
                nc.vector.max_index(
                    out=idxs[:, n, :], in_max=ones[:], in_values=lbig[:, n, :]
                )

            H = D // 2

            def load_x(n):
                if n < NT - 1:
                    nc.gpsimd.dma_start(out=xbig[:, n, :], in_=x_r[:, n, :])
                else:
                    nc.gpsimd.dma_start(
                        out=xbig[:, n, 0:H], in_=x_r[:, n, 0:H]
                    )
                    nc.gpsimd.dma_start(
                        out=xbig[:, n, H:D], in_=x_r[:, n, H:D]
                    )

            def gather(n):
                nc.gpsimd.indirect_dma_start(
                    out=ctile[:, n, :],
                    out_offset=None,
                    in_=centers[:],
                    in_offset=bass.IndirectOffsetOnAxis(
                        ap=idxs[:, n, 0:1], axis=0
                    ),
                )

            # Everything else rides the single SWDGE queue FIFO: x chunks
            # lead (ready immediately; the brief mix with the labels stream
            # beats letting the queue idle), gathers slot in as the index
            # chain delivers offsets. Pair (x_n, gather_n) completes in
            # FIFO order so sub_n/square_n pipeline behind the queue.
            load_x(0)
            load_x(1)
            load_x(2)
            for n in range(NT):
                gather(n)
                if n + 3 < NT:
                    load_x(n + 3)

            # sub -> square+accum per tile, trailing the loads
            for n in range(NT - 1):
                dif = dif_a if n % 2 == 0 else dif_b
                nc.vector.tensor_sub(
                    out=dif[:], in0=xbig[:, n, :], in1=ctile[:, n, :]
                )
                nc.scalar.activation(
                    out=dsq[:],
                    in_=dif[:],
                    func=mybir.ActivationFunctionType.Square,
                    accum_out=acc[:, n:n + 1],
                )
            for h in range(2):
                sl = slice(h * H, (h + 1) * H)
                dif = dif_b if h == 0 else dif_a
                nc.vector.tensor_sub(
                    out=dif[:, sl], in0=xbig[:, NT - 1, sl], in1=ctile[:, NT - 1, sl]
                )
                nc.scalar.activation(
                    out=dsq[:, sl],
                    in_=dif[:, sl],
                    func=mybir.ActivationFunctionType.Square,
                    accum_out=acc[:, NT - 1 + h:NT + h],
                )

            # out store on the warm SWDGE queue (cold HWDGE rings cost ~4us
            # on first use)
            nc.gpsimd.dma_start(out=out[:], in_=acc[:])

    nc.compile()
    return nc


_NC = None


def _get_nc():
    global _NC
    if _NC is None:
        _NC = build_nc()
    return _NC


def run_sharded(inputs: dict, trace: bool = False):
    """Shard, run on 8 cores, return (per_sample [B] f32, BassKernelResults)."""
    x = np.ascontiguousarray(np.asarray(inputs["x"], dtype=np.float32))
    labels = np.ascontiguousarray(np.asarray(inputs["labels"], dtype=np.float32))
    centers = np.ascontiguousarray(np.asarray(inputs["centers"], dtype=np.float32))
    assert x.shape == (B, D) and labels.shape == (B, C) and centers.shape == (C, D)

    in_maps = [
        {
            "x": np.ascontiguousarray(x[k * BS:(k + 1) * BS]),
            "labels": np.ascontiguousarray(labels[k * BS:(k + 1) * BS]),
            "centers": centers,
        }
        for k in range(NCORES)
    ]
    res = run_bass_kernel_spmd(
        _get_nc(), in_maps, core_ids=list(range(NCORES)), trace=trace
    )
    # out[p, n] holds sample k*BS + n*P + p; cols 7 and 8 are the two
    # half-sums of tile 7
    per_sample = np.concatenate(
        [
            (lambda o: np.concatenate([o[:, :NT - 1], (o[:, NT - 1:NT] + o[:, NT:NT + 1])], axis=1))(
                res.results[k]["out"]
            ).T.reshape(-1)
            for k in range(NCORES)
        ]
    )
    return per_sample, res


def kernel(x, labels, centers):
    per_sample, _ = run_sharded({"x": x, "labels": labels, "centers": centers})
    per_sample = np.clip(per_sample, CLIP_LO, CLIP_HI)
    return np.asarray(per_sample.mean(dtype=np.float64), dtype=np.float32)


# revision 11
# speedup vs baseline: 1.0738x; 1.0738x over previous
"""CenterLoss kernel for Trainium2 (8 NeuronCores, data-parallel over batch).

reference:  mean(clip(rowsum((x - labels @ centers)^2), 1e-12, 1e12))
labels are exact one-hot rows, so labels @ centers is an embedding gather:
    idx[b]  = max_index(labels[b, :])           (DVE max_index, query = 1.0)
    c[b]    = centers[idx[b], :]                (indirect DMA row gather)
    ps[b]   = rowsum((x[b] - c[b])^2)           (DVE sub -> bf16, ACT square+accum)
Per-core output is a [128, 9] tile of per-sample sums (tile 7 split in two
halves for a shorter tail); the host merges the halves, applies the clip
(never binding for this data, but exact) and takes the mean.

Phase schedule (SWDGE + HWDGE queues running concurrently drop aggregate
DMA from ~430 to ~320 GB/s, so the phases are serialized):
  1. labels on the sync HWDGE ring (3MB, lands ~14.5us)
  2. index chain (DVE FIND_INDEX8) + row gathers on the SWDGE queue
     (8MB, solo, lands ~33us)
  3. x on the sync ring (8MB, solo), gated behind the last gather by a
     WAW write into xbig; 1MB chunks so sub/ACT pipeline per tile
  4. sub -> square+accum trail the x chunks; out store on the scalar
     ring right after the last accumulator read (same-engine ordering)
"""

import numpy as np

import concourse.bacc as bacc
import concourse.bass as bass
import concourse.mybir as mybir
from concourse.tile import TileContext
from concourse.bass_utils import run_bass_kernel_spmd

F32 = mybir.dt.float32
BF16 = mybir.dt.bfloat16
U32 = mybir.dt.uint32

NCORES = 8
B = 8192          # full batch
C = 751           # num classes
D = 2048          # feature dim
BS = B // NCORES  # batch per core = 1024
P = 128           # partitions
NT = BS // P      # batch tiles per core = 8
NACC = NT + 1     # tile 7 contributes two half-sums

CLIP_LO, CLIP_HI = 1e-12, 1e12


def build_nc():
    nc = bacc.Bacc(
        "TRN2",
        target_bir_lowering=False,
        debug=False,
        num_devices=NCORES,
    )
    x = nc.dram_tensor("x", [BS, D], F32, kind="ExternalInput")
    labels = nc.dram_tensor("labels", [BS, C], F32, kind="ExternalInput")
    centers = nc.dram_tensor("centers", [C, D], F32, kind="ExternalInput")
    out = nc.dram_tensor("out", [P, NACC], F32, kind="ExternalOutput")

    with TileContext(nc) as tc:
        with (
            tc.tile_pool(name="big", bufs=1) as bigpool,
            tc.tile_pool(name="small", bufs=1) as spool,
        ):
            ones = spool.tile([P, 8], F32)
            idxs = spool.tile([P, NT, 8], U32)
            acc = spool.tile([P, NACC], F32)
            dif_a = spool.tile([P, D], BF16)
            dif_b = spool.tile([P, D], BF16)
            dsq = spool.tile([P, D], BF16)
            lbig = bigpool.tile([P, NT, C], F32)
            xbig = bigpool.tile([P, NT, D], F32)
            ctile = bigpool.tile([P, NT, D], F32)

            nc.vector.memset(ones[:], 1.0)

            labels_r = labels.rearrange("(n p) c -> p n c", p=P)
            x_r = x.rearrange("(n p) d -> p n d", p=P)

            # labels on the sync HWDGE ring: solo while the SWDGE queue
            # warms up, and HWDGE completion latency is ~3us shorter, so
            # the index chain starts early
            for g in range(2):
                nc.sync.dma_start(
                    out=lbig[:, 4 * g:4 * g + 4, :],
                    in_=labels_r[:, 4 * g:4 * g + 4, :],
                )

            # one-hot -> index (DVE)
            for n in range(NT):
                nc.vector.max_index(
                    out=idxs[:, n, :], in_max=ones[:], in_values=lbig[:, n, :]
                )

            H = D // 2

            def load_x(n):
                if n < NT - 1:
                    nc.gpsimd.dma_start(out=xbig[:, n, :], in_=x_r[:, n, :])
                else:
                    nc.gpsimd.dma_start(
                        out=xbig[:, n, 0:H], in_=x_r[:, n, 0:H]
                    )
                    nc.gpsimd.dma_start(
                        out=xbig[:, n, H:D], in_=x_r[:, n, H:D]
                    )

            def gather(n):
                nc.gpsimd.indirect_dma_start(
                    out=ctile[:, n, :],
                    out_offset=None,
                    in_=centers[:],
                    in_offset=bass.IndirectOffsetOnAxis(
                        ap=idxs[:, n, 0:1], axis=0
                    ),
                )

            # Gathers then x, all on the single SWDGE queue FIFO. The queue
            # idles while labels stream on the sync ring (gathers are not
            # ready yet), then runs solo at full rate: gathers 8MB, x 8MB.
            # The sub/square pipeline trails the x chunks.
            for n in range(NT):
                gather(n)
            for n in range(NT):
                load_x(n)

            # sub -> square+accum per tile, trailing the loads
            for n in range(NT - 1):
                dif = dif_a if n % 2 == 0 else dif_b
                nc.vector.tensor_sub(
                    out=dif[:], in0=xbig[:, n, :], in1=ctile[:, n, :]
                )
                nc.scalar.activation(
                    out=dsq[:],
                    in_=dif[:],
                    func=mybir.ActivationFunctionType.Square,
                    accum_out=acc[:, n:n + 1],
                )
            for h in range(2):
                sl = slice(h * H, (h + 1) * H)
                dif = dif_b if h == 0 else dif_a
                nc.vector.tensor_sub(
                    out=dif[:, sl], in0=xbig[:, NT - 1, sl], in1=ctile[:, NT - 1, sl]
                )
                nc.scalar.activation(
                    out=dsq[:, sl],
                    in_=dif[:, sl],
                    func=mybir.ActivationFunctionType.Square,
                    accum_out=acc[:, NT - 1 + h:NT + h],
                )

            # out store on the warm SWDGE queue (cold HWDGE rings cost ~4us
            # on first use)
            nc.gpsimd.dma_start(out=out[:], in_=acc[:])

    nc.compile()
    return nc


_NC = None


def _get_nc():
    global _NC
    if _NC is None:
        _NC = build_nc()
    return _NC


def run_sharded(inputs: dict, trace: bool = False):
    """Shard, run on 8 cores, return (per_sample [B] f32, BassKernelResults)."""
    x = np.ascontiguousarray(np.asarray(inputs["x"], dtype=np.float32))
    labels = np.ascontiguousarray(np.asarray(inputs["labels"], dtype=np.float32))
    centers = np.ascontiguousarray(np.asarray(inputs["centers"], dtype=np.float32))
    assert x.shape == (B, D) and labels.shape == (B, C) and centers.shape == (C, D)

    in_maps = [
        {
            "x": np.ascontiguousarray(x[k * BS:(k + 1) * BS]),
            "labels": np.ascontiguousarray(labels[k * BS:(k + 1) * BS]),
            "centers": centers,
        }
        for k in range(NCORES)
    ]
    res = run_bass_kernel_spmd(
        _get_nc(), in_maps, core_ids=list(range(NCORES)), trace=trace
    )
    # out[p, n] holds sample k*BS + n*P + p; cols 7 and 8 are the two
    # half-sums of tile 7
    per_sample = np.concatenate(
        [
            (lambda o: np.concatenate([o[:, :NT - 1], (o[:, NT - 1:NT] + o[:, NT:NT + 1])], axis=1))(
                res.results[k]["out"]
            ).T.reshape(-1)
            for k in range(NCORES)
        ]
    )
    return per_sample, res


def kernel(x, labels, centers):
    per_sample, _ = run_sharded({"x": x, "labels": labels, "centers": centers})
    per_sample = np.clip(per_sample, CLIP_LO, CLIP_HI)
    return np.asarray(per_sample.mean(dtype=np.float64), dtype=np.float32)
